# revision 58
# baseline (speedup 1.0000x reference)
"""CRF negative-log-likelihood loss kernel for Trainium2 (8 NeuronCores).

Problem: nn_ConditionalRandomField — B=128, S=512, T=256.
loss = mean_b( log Z_b - score_b ).

Strategy (data-parallel over batch, 16 batches/core):
  * Partition function in exp space, no renormalization (c=1/422
    prescale keeps q in range over 511 steps; ln c re-added at end).
  * The serial scan is split in HALF: a forward recurrence
    qf(s) = e_s (*) (A^T qf(s-1)) over steps 1..255 and a backward
    recurrence xb(s) = e_s (*) (A xb(s+1)) over steps 511..257 run as
    two independent chains that fill each other's engine-latency gaps
    (PE matmul block -> DVE PSUM-multiply round trip). They meet at
    Z_b = qf(255)^T . A xb(256-ish), one matmul + multiply at the end.
    This halves the number of serial rounds (255 instead of 511).
  * u PSUM tiles are double-buffered so the first matmul of each round
    carries only the DVE-data wait — its LDWEIGHTS issues during the
    multiply instead of after it.
  * Emission prep (PE transpose -> ACT exp into the resident eem
    buffer) and the gold-path score (DVE one-hots -> accumulated count
    matmuls with [onehot_next | em_bf16] moving; Pool casts) are
    spread as fine-grained quanta through the scan rounds.

Self-contained: shapes/sharding hardcoded.
"""

import math
import numpy as np

_B, _S, _T = 128, 512, 256
_NCORES = 8
_BL = _B // _NCORES          # 16 batches per core
_NCH = _S // 128             # 4 chunks of 128 steps
_CDEN = 422.0
_LN_CDEN = math.log(_CDEN)
_HALF = 255                  # fwd rounds; bwd does 255 + 1 closing MM

_cache = {}
last_results = None


def _build_program():
    from contextlib import ExitStack

    import concourse.bass as bass
    import concourse.tile as tile
    from concourse import bacc, mybir

    f32 = mybir.dt.float32
    bf16 = mybir.dt.bfloat16
    i32 = mybir.dt.int32
    MUL = mybir.AluOpType.mult
    ADD = mybir.AluOpType.add
    SUB = mybir.AluOpType.subtract
    EQ = mybir.AluOpType.is_equal
    EXP = mybir.ActivationFunctionType.Exp
    LN = mybir.ActivationFunctionType.Ln
    X = mybir.AxisListType.X

    nc = bacc.Bacc("TRN2", target_bir_lowering=False, debug=False,
                   num_devices=_NCORES)

    em_d = nc.dram_tensor("em", [_BL, _S, _T], f32, kind="ExternalInput")
    tags_d = nc.dram_tensor("tags", [_BL, _S], i32, kind="ExternalInput")
    trans_d = nc.dram_tensor("trans", [_T, _T], f32, kind="ExternalInput")
    start_d = nc.dram_tensor("start_t", [_T], f32, kind="ExternalInput")
    end_d = nc.dram_tensor("end_t", [_T], f32, kind="ExternalInput")
    part_d = nc.dram_tensor("partial", [1, 1], f32, kind="ExternalOutput")

    with tile.TileContext(nc) as tc, ExitStack() as ctx:
        singles = ctx.enter_context(tc.tile_pool(name="singles", bufs=1))

        # ---- constants ----
        iota_i = singles.tile([128, _T], i32)
        nc.gpsimd.iota(iota_i[:], pattern=[[1, _T]], base=0, channel_multiplier=0)
        iota_f = singles.tile([128, _T], f32)
        nc.vector.tensor_copy(iota_f[:], iota_i[:])
        pidx_i = singles.tile([128, 1], i32)
        nc.gpsimd.iota(pidx_i[:], pattern=[[0, 1]], base=0, channel_multiplier=1)
        pidx_f = singles.tile([128, 1], f32)
        nc.vector.tensor_copy(pidx_f[:], pidx_i[:])
        ident = singles.tile([128, 128], f32)
        nc.vector.tensor_scalar(out=ident[:], in0=iota_f[:, 0:128],
                                scalar1=pidx_f[:, 0:1], scalar2=None, op0=EQ)
        ones_f = singles.tile([128, 1], f32)
        nc.vector.memset(ones_f[:], 1.0)
        ones_row = singles.tile([1, 128], f32)
        nc.vector.memset(ones_row[:], 1.0)
        lnc_neg = singles.tile([128, 1], f32)
        nc.vector.memset(lnc_neg[:], -_LN_CDEN)
        lnc_pos = singles.tile([128, 1], f32)
        nc.vector.memset(lnc_pos[:], _LN_CDEN)

        # ---- pools ----
        empool = ctx.enter_context(tc.tile_pool(name="em", bufs=33))
        ohpool = ctx.enter_context(tc.tile_pool(name="oh", bufs=4))
        mvpool = ctx.enter_context(tc.tile_pool(name="mv", bufs=4))
        tppool = ctx.enter_context(
            tc.tile_pool(name="tp", bufs=2, space="PSUM"))
        cpool = ctx.enter_context(
            tc.tile_pool(name="cps", bufs=1, space="PSUM"))
        ufpool = ctx.enter_context(
            tc.tile_pool(name="uf", bufs=2, space="PSUM"))
        ubpool = ctx.enter_context(
            tc.tile_pool(name="ub", bufs=2, space="PSUM"))
        qpool = ctx.enter_context(tc.tile_pool(name="q", bufs=2))

        # ---- transition params (A = exp(trans)) ----
        tr_sb = singles.tile([128, 2, _T], f32)
        nc.sync.dma_start(tr_sb[:], trans_d[:].rearrange("(h p) j -> p h j", p=128))
        etrans = singles.tile([128, 2, _T], bf16)
        nc.scalar.activation(etrans[:, 0, :], tr_sb[:, 0, :], EXP, bias=0.0, scale=1.0)
        nc.scalar.activation(etrans[:, 1, :], tr_sb[:, 1, :], EXP, bias=0.0, scale=1.0)
        # transposed A for the backward chain: eAT[p, hj, i] = exp(trans[i, hj*128+p])
        eAT = singles.tile([128, 2, _T], bf16)
        for hi in range(2):
            for hj in range(2):
                tpx = tppool.tile([128, 128], f32, tag="tp")
                nc.tensor.transpose(tpx[:], tr_sb[:, hi, hj * 128:(hj + 1) * 128],
                                    ident[:])
                nc.scalar.activation(eAT[:, hj, hi * 128:(hi + 1) * 128], tpx[:],
                                     EXP, bias=0.0, scale=1.0)

        st_pc = singles.tile([128, 2], f32)
        nc.sync.dma_start(st_pc[:], start_d[:].rearrange("(h p) -> p h", p=128))
        estart = singles.tile([128, 2], f32)  # exp(start)/c
        nc.scalar.activation(estart[:], st_pc[:], EXP, bias=lnc_pos[:, 0:1], scale=1.0)
        en_pc = singles.tile([128, 2], f32)
        nc.sync.dma_start(en_pc[:], end_d[:].rearrange("(h p) -> p h", p=128))
        eend = singles.tile([128, 2], f32)
        nc.scalar.activation(eend[:], en_pc[:], EXP, bias=0.0, scale=1.0)
        pidx2_i = singles.tile([128, 2], i32)
        nc.gpsimd.iota(pidx2_i[:], pattern=[[128, 2]], base=0, channel_multiplier=1)
        pidx2_f = singles.tile([128, 2], f32)
        nc.vector.tensor_copy(pidx2_f[:], pidx2_i[:])

        # ---- tag columns (tiles; DMAs emitted after the em bootstrap) ----
        tcol_i = singles.tile([128, _BL, _NCH], i32)
        tcol2_i = singles.tile([128, _BL, _NCH], i32)
        nc.gpsimd.memset(tcol2_i[:], -1)
        tcol_f = singles.tile([128, _BL, _NCH], f32)
        tcol2_f = singles.tile([128, _BL, _NCH], f32)
        tf_i = singles.tile([1, _BL], i32)
        tl_i = singles.tile([1, _BL], i32)
        oh_se = singles.tile([128, 2, 2, _BL], f32)

        def emit_tag_dmas():
            nc.sync.dma_start(tcol_i[:],
                              tags_d[:].rearrange("b (c p) -> p b c", p=128))
            for b in range(_BL):
                nc.sync.dma_start(
                    tcol2_i[:, b, 0:_NCH - 1],
                    tags_d[b, 1:1 + 128 * (_NCH - 1)].rearrange("(c p) -> p c", p=128))
                nc.sync.dma_start(
                    tcol2_i[0:127, b, _NCH - 1:_NCH],
                    tags_d[b, 1 + 128 * (_NCH - 1):_S].rearrange("(c p) -> p c", p=127))
            nc.sync.dma_start(tf_i[:], tags_d[:, 0:1].rearrange("b o -> o b"))
            nc.sync.dma_start(tl_i[:], tags_d[:, _S - 1:_S].rearrange("b o -> o b"))

        rnum = singles.tile([128, 6], f32)
        # one eem tile per 128-step chunk so a scan round's multiply only
        # depends on its own chunk's exp writes (Tile's range tracking
        # would otherwise serialize every round behind every exp)
        eem0 = singles.tile([128, 2, _BL, 128], bf16)
        eem1 = singles.tile([128, 2, _BL, 128], bf16)
        eem2 = singles.tile([128, 2, _BL, 128], bf16)
        eem3 = singles.tile([128, 2, _BL, 128], bf16)
        eem_t = [eem0, eem1, eem2, eem3]
        c_ps = cpool.tile([128, 2, 2 * _T], f32)

        # ---------- prep pieces ----------
        emt = {}

        def dma_chunk_pair(b, cp, eng):
            """One DMA loads chunks 2*cp and 2*cp+1 for batch b."""
            t = empool.tile([128, 2, _T], f32, tag="emt")
            eng.dma_start(t[:], em_d[b, cp * 256:(cp + 1) * 256, :]
                          .rearrange("(c p) t -> p c t", p=128))
            emt[(b, 2 * cp)] = (t, 0)
            emt[(b, 2 * cp + 1)] = (t, 1)

        _tp_rr = {"i": 0}

        def tpexp_quanta(b, ch, rotate=False):
            """Transpose both tag halves into one PSUM tile, then a single
            exp writes eem for this (batch, chunk): 3 quanta. In bootstrap
            (rotate=True) PSUM slots rotate across tp/uf/ub pools."""
            st = {}

            def tp(jh):
                def go():
                    if jh == 0:
                        if rotate:
                            pool, tg = ((tppool, "tp"), (ufpool, "uf"),
                                        (ubpool, "ub"))[_tp_rr["i"] % 3]
                            _tp_rr["i"] += 1
                        else:
                            pool, tg = tppool, "tp"
                        tpt = pool.tile([128, 2, 128], f32, tag=tg)
                        st["t"] = tpt
                    et, ec = emt[(b, ch)]
                    nc.tensor.transpose(st["t"][:, jh, :],
                                        et[:, ec, jh * 128:(jh + 1) * 128],
                                        ident[:])
                return go

            def ex():
                nc.scalar.activation(
                    eem_t[ch][:, :, b, :], st.pop("t"), EXP,
                    bias=lnc_neg[:, 0:1], scale=1.0)

            return [tp(0), tp(1), ex]

        _cmm_state = {"n": 0}

        def score_quanta(b, ch):
            """One-hots + cast + count-matmuls for one (batch, chunk)."""
            st = {}

            def g_oh1():
                oh1 = ohpool.tile([128, _T], bf16, tag="oh1")
                st["oh1"] = oh1
                nc.vector.tensor_scalar(out=oh1[:], in0=iota_f[:],
                                        scalar1=tcol_f[:, b, ch:ch + 1],
                                        scalar2=None, op0=EQ)

            def g_mv():
                mv = mvpool.tile([128, 2 * _T], bf16, tag="mv")
                st["mv"] = mv
                nc.vector.tensor_scalar(out=mv[:, 0:_T], in0=iota_f[:],
                                        scalar1=tcol2_f[:, b, ch:ch + 1],
                                        scalar2=None, op0=EQ)
                et, ec = emt.pop((b, ch))
                nc.gpsimd.tensor_copy(mv[:, _T:2 * _T], et[:, ec, :])

            def g_cmm():
                first = _cmm_state["n"] == 0
                _cmm_state["n"] += 1
                last = _cmm_state["n"] == _BL * _NCH
                oh1, mv = st["oh1"], st["mv"]
                nc.tensor.matmul(c_ps[:, 0, :], oh1[:, 0:128], mv[:],
                                 start=first, stop=last, skip_group_check=True)
                nc.tensor.matmul(c_ps[:, 1, :], oh1[:, 128:256], mv[:],
                                 start=first, stop=last, skip_group_check=True)

            return [g_oh1, g_mv, g_cmm]

        # ---------- bootstrap ----------
        # All em DMAs on the otherwise-idle sync (SP) HWDGE queue; each
        # DMA carries a chunk pair, so 32 DMAs cover all four chunks.
        for b in range(_BL):
            dma_chunk_pair(b, 0, nc.sync)
            dma_chunk_pair(b, 1, nc.scalar)
        emit_tag_dmas()
        # transpose+exp for chunks 0 and 3 (needed before round 1)
        for b in range(_BL):
            for g in tpexp_quanta(b, 0, rotate=True):
                g()
            for g in tpexp_quanta(b, 3, rotate=True):
                g()

        # quanta stream fired through the scan rounds:
        #  - tp/exp for chunks 1,2 over rounds 1..110
        #  - score quanta for all chunks over rounds 1..250
        eem_q = []
        for b in range(_BL):
            eem_q.extend(tpexp_quanta(b, 1))
            eem_q.extend(tpexp_quanta(b, 2))
        def g_tcol_copies():
            nc.vector.tensor_copy(tcol_f[:], tcol_i[:])
            nc.vector.tensor_copy(tcol2_f[:], tcol2_i[:])

        sc_q = [g_tcol_copies]
        for ch in (0, 3, 1, 2):
            for b in range(_BL):
                sc_q.extend(score_quanta(b, ch))

        # ---------- init chains ----------
        qf = qpool.tile([128, 2 * _BL], bf16, tag="qf")
        for jh in range(2):
            nc.vector.tensor_scalar(
                out=qf[:, jh * _BL:(jh + 1) * _BL],
                in0=eem0[:, jh, :, 0],
                scalar1=estart[:, jh:jh + 1], scalar2=None, op0=MUL)
        xb = qpool.tile([128, 2 * _BL], bf16, tag="xb")
        for jh in range(2):
            nc.vector.tensor_scalar(
                out=xb[:, jh * _BL:(jh + 1) * _BL],
                in0=eem3[:, jh, :, 127],
                scalar1=eend[:, jh:jh + 1], scalar2=None, op0=MUL)

        # ---------- scan rounds ----------
        EEMQ_START, EEMQ_END = 5, 120
        SCQ_START, SCQ_END = 35, 250
        ei = si = 0
        for r in range(1, _HALF + 1):
            # forward MMs: uf = A^T qf
            uf = ufpool.tile([128, 2 * _BL], f32, tag="uf")
            for jh in range(2):
                o = uf[:, jh * _BL:(jh + 1) * _BL]
                nc.tensor.matmul(o, etrans[:, 0, jh * 128:(jh + 1) * 128],
                                 qf[:, 0:_BL], start=True, stop=False,
                                 skip_group_check=True)
                nc.tensor.matmul(o, etrans[:, 1, jh * 128:(jh + 1) * 128],
                                 qf[:, _BL:2 * _BL], start=False, stop=True,
                                 skip_group_check=True)
            # backward MMs: ub = A xb
            ub = ubpool.tile([128, 2 * _BL], f32, tag="ub")
            for ih in range(2):
                o = ub[:, ih * _BL:(ih + 1) * _BL]
                nc.tensor.matmul(o, eAT[:, 0, ih * 128:(ih + 1) * 128],
                                 xb[:, 0:_BL], start=True, stop=False,
                                 skip_group_check=True)
                nc.tensor.matmul(o, eAT[:, 1, ih * 128:(ih + 1) * 128],
                                 xb[:, _BL:2 * _BL], start=False, stop=True,
                                 skip_group_check=True)
            # multiplies: qf = uf * e_r ; xb = ub * e_{511-r}
            qf = qpool.tile([128, 2 * _BL], bf16, tag="qf")
            nc.vector.tensor_tensor(out=qf[:], in0=uf[:],
                                    in1=eem_t[r // 128][:, :, :, r % 128], op=MUL)
            xb = qpool.tile([128, 2 * _BL], bf16, tag="xb")
            nc.vector.tensor_tensor(out=xb[:], in0=ub[:],
                                    in1=eem_t[(_S - 1 - r) // 128]
                                    [:, :, :, (_S - 1 - r) % 128], op=MUL)
            # fire prep quanta inside their round windows
            ne = (len(eem_q) * max(0, min(r, EEMQ_END) - EEMQ_START)) \
                // (EEMQ_END - EEMQ_START) - ei
            for _ in range(ne):
                eem_q[ei]()
                ei += 1
            ns = (len(sc_q) * max(0, min(r, SCQ_END) - SCQ_START)) \
                // (SCQ_END - SCQ_START) - si
            for _ in range(ns):
                sc_q[si]()
                si += 1

        # ---------- close: qb(256) = A xb_final ; Z = qf . qb ----------
        ub = ubpool.tile([128, 2 * _BL], f32, tag="ub")
        for ih in range(2):
            o = ub[:, ih * _BL:(ih + 1) * _BL]
            nc.tensor.matmul(o, eAT[:, 0, ih * 128:(ih + 1) * 128],
                             xb[:, 0:_BL], start=True, stop=False,
                             skip_group_check=True)
            nc.tensor.matmul(o, eAT[:, 1, ih * 128:(ih + 1) * 128],
                             xb[:, _BL:2 * _BL], start=False, stop=True,
                             skip_group_check=True)
        fpool = ctx.enter_context(tc.tile_pool(name="f", bufs=1))
        zprod = fpool.tile([128, 2 * _BL], f32)
        nc.vector.tensor_tensor(out=zprod[:], in0=ub[:], in1=qf[:], op=MUL)
        zf = tppool.tile([1, 2 * _BL], f32, tag="tp")
        nc.tensor.matmul(zf[:], ones_f[:], zprod[:], start=True, stop=True,
                         skip_group_check=True)
        zsb = fpool.tile([1, 2 * _BL], f32)
        nc.vector.tensor_copy(zsb[:], zf[:])
        zsum = fpool.tile([1, _BL], f32)
        nc.vector.tensor_tensor(out=zsum[:], in0=zsb[:, 0:_BL],
                                in1=zsb[:, _BL:2 * _BL], op=ADD)
        logz = fpool.tile([1, _BL], f32)
        nc.scalar.activation(logz[:], zsum[:], LN, bias=0.0, scale=1.0)
        nc.vector.tensor_scalar(out=logz[:], in0=logz[:],
                                scalar1=float((_S - 1) * _LN_CDEN), scalar2=None,
                                op0=ADD)

        # ---------- start/end one-hot terms ----------
        tf_f = fpool.tile([1, _BL], f32)
        nc.vector.tensor_copy(tf_f[:], tf_i[:])
        tl_f = fpool.tile([1, _BL], f32)
        nc.vector.tensor_copy(tl_f[:], tl_i[:])
        for k, (srci, par) in enumerate(((tf_f, st_pc), (tl_f, en_pc))):
            se_ps = tppool.tile([128, _BL], f32, tag="tp")
            nc.tensor.matmul(se_ps[:], ones_row[:], srci[:],
                             start=True, stop=True)
            for h in range(2):
                nc.vector.tensor_scalar(out=oh_se[:, k, h, :], in0=se_ps[:],
                                        scalar1=pidx2_f[:, h:h + 1],
                                        scalar2=par[:, h:h + 1],
                                        op0=EQ, op1=MUL)
            nc.vector.tensor_reduce(rnum[:, 4 + k:5 + k],
                                    oh_se[:, k, :, :],
                                    axis=mybir.AxisListType.XY, op=ADD)

        # ---------- numerator finalization ----------
        scpool = ctx.enter_context(tc.tile_pool(name="sc", bufs=2))
        for ih in range(2):
            scr = scpool.tile([128, _T], f32, tag="scr")
            nc.vector.tensor_tensor(out=scr[:], in0=c_ps[:, ih, 0:_T],
                                    in1=tr_sb[:, ih, :], op=MUL)
            nc.vector.tensor_reduce(rnum[:, ih:ih + 1], scr[:], axis=X, op=ADD)
        for ih in range(2):
            scr = scpool.tile([128, 128], f32, tag="scr2")
            nc.vector.tensor_tensor(
                out=scr[:], in0=c_ps[:, ih, _T + ih * 128:_T + (ih + 1) * 128],
                in1=ident[:], op=MUL)
            nc.vector.tensor_reduce(rnum[:, 2 + ih:3 + ih], scr[:], axis=X, op=ADD)

        slz = fpool.tile([1, 1], f32)
        nc.vector.tensor_reduce(slz[:], logz[:], axis=X, op=ADD)
        rsum = fpool.tile([128, 1], f32)
        nc.vector.tensor_reduce(rsum[:], rnum[:], axis=X, op=ADD)
        nsum = tppool.tile([1, 1], f32, tag="tp")
        nc.tensor.matmul(nsum[:], ones_f[:], rsum[:], start=True, stop=True,
                         skip_group_check=True)
        part = fpool.tile([1, 1], f32)
        nc.vector.tensor_tensor(out=part[:], in0=slz[:], in1=nsum[:], op=SUB)
        nc.sync.dma_start(part_d[:], part[:])

    nc.compile()
    return nc


def kernel(emissions, tags, masks=None, start_transitions=None,
           transitions=None, end_transitions=None, **_unused):
    from concourse.bass_utils import run_bass_kernel_spmd

    global last_results
    nc = _cache.get("nc")
    if nc is None:
        nc = _build_program()
        _cache["nc"] = nc

    em = np.ascontiguousarray(np.asarray(emissions, dtype=np.float32))
    tg = np.ascontiguousarray(np.asarray(tags).astype(np.int32))
    tr = np.ascontiguousarray(np.asarray(transitions, dtype=np.float32))
    st = np.ascontiguousarray(np.asarray(start_transitions, dtype=np.float32))
    en = np.ascontiguousarray(np.asarray(end_transitions, dtype=np.float32))
    # masks are all ones for this problem (spec fill: "ones") — unused.

    in_maps = []
    for k in range(_NCORES):
        sl = slice(k * _BL, (k + 1) * _BL)
        in_maps.append(dict(em=em[sl], tags=tg[sl], trans=tr,
                            start_t=st, end_t=en))
    res = run_bass_kernel_spmd(nc, in_maps, list(range(_NCORES)))
    last_results = res
    total = sum(float(r["partial"][0, 0]) for r in res.results)
    return np.float32(total / _B)


# revision 59
# speedup vs baseline: 1.0802x; 1.0802x over previous
"""CRF negative-log-likelihood loss kernel for Trainium2 (8 NeuronCores).

Problem: nn_ConditionalRandomField — B=128, S=512, T=256.
loss = mean_b( log Z_b - score_b ).

Strategy (data-parallel over batch, 16 batches/core):
  * Partition function in exp space, no renormalization (c=1/422
    prescale keeps q in range over 511 steps; ln c re-added at end).
  * The serial scan is split in HALF: a forward recurrence
    qf(s) = e_s (*) (A^T qf(s-1)) over steps 1..255 and a backward
    recurrence xb(s) = e_s (*) (A xb(s+1)) over steps 511..257 run as
    two independent chains that fill each other's engine-latency gaps
    (PE matmul block -> DVE PSUM-multiply round trip). They meet at
    Z_b = qf(255)^T . A xb(256-ish), one matmul + multiply at the end.
    This halves the number of serial rounds (255 instead of 511).
  * u PSUM tiles are double-buffered so the first matmul of each round
    carries only the DVE-data wait — its LDWEIGHTS issues during the
    multiply instead of after it.
  * Emission prep (PE transpose -> ACT exp into the resident eem
    buffer) and the gold-path score (DVE one-hots -> accumulated count
    matmuls with [onehot_next | em_bf16] moving; Pool casts) are
    spread as fine-grained quanta through the scan rounds.

Self-contained: shapes/sharding hardcoded.
"""

import math
import numpy as np

_B, _S, _T = 128, 512, 256
_NCORES = 8
_BL = _B // _NCORES          # 16 batches per core
_NCH = _S // 128             # 4 chunks of 128 steps
_CDEN = 422.0
_LN_CDEN = math.log(_CDEN)
_HALF = 255                  # fwd rounds; bwd does 255 + 1 closing MM

_cache = {}
last_results = None


def _build_program():
    from contextlib import ExitStack

    import concourse.bass as bass
    import concourse.tile as tile
    from concourse import bacc, mybir

    f32 = mybir.dt.float32
    bf16 = mybir.dt.bfloat16
    i32 = mybir.dt.int32
    MUL = mybir.AluOpType.mult
    ADD = mybir.AluOpType.add
    SUB = mybir.AluOpType.subtract
    EQ = mybir.AluOpType.is_equal
    EXP = mybir.ActivationFunctionType.Exp
    LN = mybir.ActivationFunctionType.Ln
    X = mybir.AxisListType.X

    nc = bacc.Bacc("TRN2", target_bir_lowering=False, debug=False,
                   num_devices=_NCORES)

    em_d = nc.dram_tensor("em", [_BL, _S, _T], f32, kind="ExternalInput")
    tags_d = nc.dram_tensor("tags", [_BL, _S], i32, kind="ExternalInput")
    trans_d = nc.dram_tensor("trans", [_T, _T], f32, kind="ExternalInput")
    start_d = nc.dram_tensor("start_t", [_T], f32, kind="ExternalInput")
    end_d = nc.dram_tensor("end_t", [_T], f32, kind="ExternalInput")
    part_d = nc.dram_tensor("partial", [1, 1], f32, kind="ExternalOutput")

    with tile.TileContext(nc) as tc, ExitStack() as ctx:
        singles = ctx.enter_context(tc.tile_pool(name="singles", bufs=1))

        # ---- constants ----
        iota_i = singles.tile([128, _T], i32)
        nc.gpsimd.iota(iota_i[:], pattern=[[1, _T]], base=0, channel_multiplier=0)
        iota_f = singles.tile([128, _T], f32)
        nc.vector.tensor_copy(iota_f[:], iota_i[:])
        pidx_i = singles.tile([128, 1], i32)
        nc.gpsimd.iota(pidx_i[:], pattern=[[0, 1]], base=0, channel_multiplier=1)
        pidx_f = singles.tile([128, 1], f32)
        nc.vector.tensor_copy(pidx_f[:], pidx_i[:])
        ident = singles.tile([128, 128], f32)
        nc.vector.tensor_scalar(out=ident[:], in0=iota_f[:, 0:128],
                                scalar1=pidx_f[:, 0:1], scalar2=None, op0=EQ)
        ones_f = singles.tile([128, 1], f32)
        nc.vector.memset(ones_f[:], 1.0)
        ones_row = singles.tile([1, 128], f32)
        nc.vector.memset(ones_row[:], 1.0)
        lnc_neg = singles.tile([128, 1], f32)
        nc.vector.memset(lnc_neg[:], -_LN_CDEN)
        lnc_pos = singles.tile([128, 1], f32)
        nc.vector.memset(lnc_pos[:], _LN_CDEN)

        # ---- pools ----
        empool = ctx.enter_context(tc.tile_pool(name="em", bufs=33))
        ohpool = ctx.enter_context(tc.tile_pool(name="oh", bufs=4))
        mvpool = ctx.enter_context(tc.tile_pool(name="mv", bufs=4))
        tppool = ctx.enter_context(
            tc.tile_pool(name="tp", bufs=2, space="PSUM"))
        cpool = ctx.enter_context(
            tc.tile_pool(name="cps", bufs=1, space="PSUM"))
        ufpool = ctx.enter_context(
            tc.tile_pool(name="uf", bufs=2, space="PSUM"))
        ubpool = ctx.enter_context(
            tc.tile_pool(name="ub", bufs=2, space="PSUM"))
        qpool = ctx.enter_context(tc.tile_pool(name="q", bufs=2))

        # ---- transition params (A = exp(trans)) ----
        tr_sb = singles.tile([128, 2, _T], f32)
        nc.sync.dma_start(tr_sb[:], trans_d[:].rearrange("(h p) j -> p h j", p=128))
        etrans = singles.tile([128, 2, _T], bf16)
        nc.scalar.activation(etrans[:, 0, :], tr_sb[:, 0, :], EXP, bias=0.0, scale=1.0)
        nc.scalar.activation(etrans[:, 1, :], tr_sb[:, 1, :], EXP, bias=0.0, scale=1.0)
        # transposed A for the backward chain: eAT[p, hj, i] = exp(trans[i, hj*128+p])
        eAT = singles.tile([128, 2, _T], bf16)
        for hi in range(2):
            for hj in range(2):
                tpx = tppool.tile([128, 128], f32, tag="tp")
                nc.tensor.transpose(tpx[:], tr_sb[:, hi, hj * 128:(hj + 1) * 128],
                                    ident[:])
                nc.scalar.activation(eAT[:, hj, hi * 128:(hi + 1) * 128], tpx[:],
                                     EXP, bias=0.0, scale=1.0)

        st_pc = singles.tile([128, 2], f32)
        nc.sync.dma_start(st_pc[:], start_d[:].rearrange("(h p) -> p h", p=128))
        estart = singles.tile([128, 2], f32)  # exp(start)/c
        nc.scalar.activation(estart[:], st_pc[:], EXP, bias=lnc_pos[:, 0:1], scale=1.0)
        en_pc = singles.tile([128, 2], f32)
        nc.sync.dma_start(en_pc[:], end_d[:].rearrange("(h p) -> p h", p=128))
        eend = singles.tile([128, 2], f32)
        nc.scalar.activation(eend[:], en_pc[:], EXP, bias=0.0, scale=1.0)
        pidx2_i = singles.tile([128, 2], i32)
        nc.gpsimd.iota(pidx2_i[:], pattern=[[128, 2]], base=0, channel_multiplier=1)
        pidx2_f = singles.tile([128, 2], f32)
        nc.vector.tensor_copy(pidx2_f[:], pidx2_i[:])

        # ---- tag columns (tiles; DMAs emitted after the em bootstrap) ----
        tcol_i = singles.tile([128, _BL, _NCH], i32)
        tcol2_i = singles.tile([128, _BL, _NCH], i32)
        nc.gpsimd.memset(tcol2_i[:], -1)
        tcol_f = singles.tile([128, _BL, _NCH], f32)
        tcol2_f = singles.tile([128, _BL, _NCH], f32)
        tf_i = singles.tile([1, _BL], i32)
        tl_i = singles.tile([1, _BL], i32)
        oh_se = singles.tile([128, 2, 2, _BL], f32)

        def emit_tag_dmas():
            nc.sync.dma_start(tcol_i[:],
                              tags_d[:].rearrange("b (c p) -> p b c", p=128))
            for b in range(_BL):
                nc.sync.dma_start(
                    tcol2_i[:, b, 0:_NCH - 1],
                    tags_d[b, 1:1 + 128 * (_NCH - 1)].rearrange("(c p) -> p c", p=128))
                nc.sync.dma_start(
                    tcol2_i[0:127, b, _NCH - 1:_NCH],
                    tags_d[b, 1 + 128 * (_NCH - 1):_S].rearrange("(c p) -> p c", p=127))
            nc.sync.dma_start(tf_i[:], tags_d[:, 0:1].rearrange("b o -> o b"))
            nc.sync.dma_start(tl_i[:], tags_d[:, _S - 1:_S].rearrange("b o -> o b"))

        rnum = singles.tile([128, 6], f32)
        # one eem tile per 128-step chunk so a scan round's multiply only
        # depends on its own chunk's exp writes (Tile's range tracking
        # would otherwise serialize every round behind every exp)
        eem0 = singles.tile([128, 2, _BL, 128], bf16)
        eem1 = singles.tile([128, 2, _BL, 128], bf16)
        eem2 = singles.tile([128, 2, _BL, 128], bf16)
        eem3 = singles.tile([128, 2, _BL, 128], bf16)
        eem_t = [eem0, eem1, eem2, eem3]
        c_ps = cpool.tile([128, 2, 2 * _T], f32)

        # ---------- prep pieces ----------
        emt = {}

        def dma_chunk_pair(b, cp, eng):
            """One DMA loads chunks 2*cp and 2*cp+1 for batch b."""
            t = empool.tile([128, 2, _T], f32, tag="emt")
            eng.dma_start(t[:], em_d[b, cp * 256:(cp + 1) * 256, :]
                          .rearrange("(c p) t -> p c t", p=128))
            emt[(b, 2 * cp)] = (t, 0)
            emt[(b, 2 * cp + 1)] = (t, 1)

        _tp_rr = {"i": 0}

        def tpexp_quanta(b, ch, rotate=False):
            """Transpose both tag halves into one PSUM tile, then a single
            exp writes eem for this (batch, chunk): 3 quanta. In bootstrap
            (rotate=True) PSUM slots rotate across tp/uf/ub pools."""
            st = {}

            def tp(jh):
                def go():
                    if jh == 0:
                        if rotate:
                            pool, tg = ((tppool, "tp"), (ufpool, "uf"),
                                        (ubpool, "ub"))[_tp_rr["i"] % 3]
                            _tp_rr["i"] += 1
                        else:
                            pool, tg = tppool, "tp"
                        tpt = pool.tile([128, 2, 128], f32, tag=tg)
                        st["t"] = tpt
                    et, ec = emt[(b, ch)]
                    nc.tensor.transpose(st["t"][:, jh, :],
                                        et[:, ec, jh * 128:(jh + 1) * 128],
                                        ident[:])
                return go

            def ex():
                nc.scalar.activation(
                    eem_t[ch][:, :, b, :], st.pop("t"), EXP,
                    bias=lnc_neg[:, 0:1], scale=1.0)

            return [tp(0), tp(1), ex]

        _cmm_state = {"n": 0}

        def score_quanta(b, ch):
            """One-hots + cast + count-matmuls for one (batch, chunk)."""
            st = {}

            def g_oh1():
                oh1 = ohpool.tile([128, _T], bf16, tag="oh1")
                st["oh1"] = oh1
                nc.vector.tensor_scalar(out=oh1[:], in0=iota_f[:],
                                        scalar1=tcol_f[:, b, ch:ch + 1],
                                        scalar2=None, op0=EQ)

            def g_mv():
                mv = mvpool.tile([128, 2 * _T], bf16, tag="mv")
                st["mv"] = mv
                nc.vector.tensor_scalar(out=mv[:, 0:_T], in0=iota_f[:],
                                        scalar1=tcol2_f[:, b, ch:ch + 1],
                                        scalar2=None, op0=EQ)
                et, ec = emt.pop((b, ch))
                nc.gpsimd.tensor_copy(mv[:, _T:2 * _T], et[:, ec, :])

            def g_cmm():
                first = _cmm_state["n"] == 0
                _cmm_state["n"] += 1
                last = _cmm_state["n"] == _BL * _NCH
                oh1, mv = st["oh1"], st["mv"]
                nc.tensor.matmul(c_ps[:, 0, :], oh1[:, 0:128], mv[:],
                                 start=first, stop=last, skip_group_check=True)
                nc.tensor.matmul(c_ps[:, 1, :], oh1[:, 128:256], mv[:],
                                 start=first, stop=last, skip_group_check=True)

            return [g_oh1, g_mv, g_cmm]

        # ---------- bootstrap ----------
        # All em DMAs on the otherwise-idle sync (SP) HWDGE queue; each
        # DMA carries a chunk pair, so 32 DMAs cover all four chunks.
        for b in range(_BL):
            dma_chunk_pair(b, 0, nc.sync)
            dma_chunk_pair(b, 1, nc.sync)
        emit_tag_dmas()
        # transpose+exp for chunks 0 and 3 (needed before round 1)
        for b in range(_BL):
            for g in tpexp_quanta(b, 0, rotate=True):
                g()
            for g in tpexp_quanta(b, 3, rotate=True):
                g()

        # quanta stream fired through the scan rounds:
        #  - tp/exp for chunks 1,2 over rounds 1..110
        #  - score quanta for all chunks over rounds 1..250
        eem_q = []
        for b in range(_BL):
            eem_q.extend(tpexp_quanta(b, 1))
            eem_q.extend(tpexp_quanta(b, 2))
        def g_tcol_copies():
            nc.vector.tensor_copy(tcol_f[:], tcol_i[:])
            nc.vector.tensor_copy(tcol2_f[:], tcol2_i[:])

        sc_q = [g_tcol_copies]
        for ch in (0, 3, 1, 2):
            for b in range(_BL):
                sc_q.extend(score_quanta(b, ch))

        # ---------- init chains ----------
        qf = qpool.tile([128, 2 * _BL], bf16, tag="qf")
        for jh in range(2):
            nc.vector.tensor_scalar(
                out=qf[:, jh * _BL:(jh + 1) * _BL],
                in0=eem0[:, jh, :, 0],
                scalar1=estart[:, jh:jh + 1], scalar2=None, op0=MUL)
        xb = qpool.tile([128, 2 * _BL], bf16, tag="xb")
        for jh in range(2):
            nc.vector.tensor_scalar(
                out=xb[:, jh * _BL:(jh + 1) * _BL],
                in0=eem3[:, jh, :, 127],
                scalar1=eend[:, jh:jh + 1], scalar2=None, op0=MUL)

        # ---------- scan rounds ----------
        EEMQ_START, EEMQ_END = 5, 120
        SCQ_START, SCQ_END = 35, 250
        ei = si = 0
        for r in range(1, _HALF + 1):
            # forward MMs: uf = A^T qf
            uf = ufpool.tile([128, 2 * _BL], f32, tag="uf")
            for jh in range(2):
                o = uf[:, jh * _BL:(jh + 1) * _BL]
                nc.tensor.matmul(o, etrans[:, 0, jh * 128:(jh + 1) * 128],
                                 qf[:, 0:_BL], start=True, stop=False,
                                 skip_group_check=True)
                nc.tensor.matmul(o, etrans[:, 1, jh * 128:(jh + 1) * 128],
                                 qf[:, _BL:2 * _BL], start=False, stop=True,
                                 skip_group_check=True)
            # backward MMs: ub = A xb
            ub = ubpool.tile([128, 2 * _BL], f32, tag="ub")
            for ih in range(2):
                o = ub[:, ih * _BL:(ih + 1) * _BL]
                nc.tensor.matmul(o, eAT[:, 0, ih * 128:(ih + 1) * 128],
                                 xb[:, 0:_BL], start=True, stop=False,
                                 skip_group_check=True)
                nc.tensor.matmul(o, eAT[:, 1, ih * 128:(ih + 1) * 128],
                                 xb[:, _BL:2 * _BL], start=False, stop=True,
                                 skip_group_check=True)
            # multiplies: qf = uf * e_r ; xb = ub * e_{511-r}
            qf = qpool.tile([128, 2 * _BL], bf16, tag="qf")
            nc.vector.tensor_tensor(out=qf[:], in0=uf[:],
                                    in1=eem_t[r // 128][:, :, :, r % 128], op=MUL)
            xb = qpool.tile([128, 2 * _BL], bf16, tag="xb")
            nc.vector.tensor_tensor(out=xb[:], in0=ub[:],
                                    in1=eem_t[(_S - 1 - r) // 128]
                                    [:, :, :, (_S - 1 - r) % 128], op=MUL)
            # fire prep quanta inside their round windows
            ne = (len(eem_q) * max(0, min(r, EEMQ_END) - EEMQ_START)) \
                // (EEMQ_END - EEMQ_START) - ei
            for _ in range(ne):
                eem_q[ei]()
                ei += 1
            ns = (len(sc_q) * max(0, min(r, SCQ_END) - SCQ_START)) \
                // (SCQ_END - SCQ_START) - si
            for _ in range(ns):
                sc_q[si]()
                si += 1

        # ---------- close: qb(256) = A xb_final ; Z = qf . qb ----------
        ub = ubpool.tile([128, 2 * _BL], f32, tag="ub")
        for ih in range(2):
            o = ub[:, ih * _BL:(ih + 1) * _BL]
            nc.tensor.matmul(o, eAT[:, 0, ih * 128:(ih + 1) * 128],
                             xb[:, 0:_BL], start=True, stop=False,
                             skip_group_check=True)
            nc.tensor.matmul(o, eAT[:, 1, ih * 128:(ih + 1) * 128],
                             xb[:, _BL:2 * _BL], start=False, stop=True,
                             skip_group_check=True)
        fpool = ctx.enter_context(tc.tile_pool(name="f", bufs=1))
        zprod = fpool.tile([128, 2 * _BL], f32)
        nc.vector.tensor_tensor(out=zprod[:], in0=ub[:], in1=qf[:], op=MUL)
        zf = tppool.tile([1, 2 * _BL], f32, tag="tp")
        nc.tensor.matmul(zf[:], ones_f[:], zprod[:], start=True, stop=True,
                         skip_group_check=True)
        zsb = fpool.tile([1, 2 * _BL], f32)
        nc.vector.tensor_copy(zsb[:], zf[:])
        zsum = fpool.tile([1, _BL], f32)
        nc.vector.tensor_tensor(out=zsum[:], in0=zsb[:, 0:_BL],
                                in1=zsb[:, _BL:2 * _BL], op=ADD)
        logz = fpool.tile([1, _BL], f32)
        nc.scalar.activation(logz[:], zsum[:], LN, bias=0.0, scale=1.0)
        nc.vector.tensor_scalar(out=logz[:], in0=logz[:],
                                scalar1=float((_S - 1) * _LN_CDEN), scalar2=None,
                                op0=ADD)

        # ---------- start/end one-hot terms ----------
        tf_f = fpool.tile([1, _BL], f32)
        nc.vector.tensor_copy(tf_f[:], tf_i[:])
        tl_f = fpool.tile([1, _BL], f32)
        nc.vector.tensor_copy(tl_f[:], tl_i[:])
        for k, (srci, par) in enumerate(((tf_f, st_pc), (tl_f, en_pc))):
            se_ps = tppool.tile([128, _BL], f32, tag="tp")
            nc.tensor.matmul(se_ps[:], ones_row[:], srci[:],
                             start=True, stop=True)
            for h in range(2):
                nc.vector.tensor_scalar(out=oh_se[:, k, h, :], in0=se_ps[:],
                                        scalar1=pidx2_f[:, h:h + 1],
                                        scalar2=par[:, h:h + 1],
                                        op0=EQ, op1=MUL)
            nc.vector.tensor_reduce(rnum[:, 4 + k:5 + k],
                                    oh_se[:, k, :, :],
                                    axis=mybir.AxisListType.XY, op=ADD)

        # ---------- numerator finalization ----------
        scpool = ctx.enter_context(tc.tile_pool(name="sc", bufs=2))
        for ih in range(2):
            scr = scpool.tile([128, _T], f32, tag="scr")
            nc.vector.tensor_tensor(out=scr[:], in0=c_ps[:, ih, 0:_T],
                                    in1=tr_sb[:, ih, :], op=MUL)
            nc.vector.tensor_reduce(rnum[:, ih:ih + 1], scr[:], axis=X, op=ADD)
        for ih in range(2):
            scr = scpool.tile([128, 128], f32, tag="scr2")
            nc.vector.tensor_tensor(
                out=scr[:], in0=c_ps[:, ih, _T + ih * 128:_T + (ih + 1) * 128],
                in1=ident[:], op=MUL)
            nc.vector.tensor_reduce(rnum[:, 2 + ih:3 + ih], scr[:], axis=X, op=ADD)

        slz = fpool.tile([1, 1], f32)
        nc.vector.tensor_reduce(slz[:], logz[:], axis=X, op=ADD)
        rsum = fpool.tile([128, 1], f32)
        nc.vector.tensor_reduce(rsum[:], rnum[:], axis=X, op=ADD)
        nsum = tppool.tile([1, 1], f32, tag="tp")
        nc.tensor.matmul(nsum[:], ones_f[:], rsum[:], start=True, stop=True,
                         skip_group_check=True)
        part = fpool.tile([1, 1], f32)
        nc.vector.tensor_tensor(out=part[:], in0=slz[:], in1=nsum[:], op=SUB)
        nc.sync.dma_start(part_d[:], part[:])

    nc.compile()
    return nc


def kernel(emissions, tags, masks=None, start_transitions=None,
           transitions=None, end_transitions=None, **_unused):
    from concourse.bass_utils import run_bass_kernel_spmd

    global last_results
    nc = _cache.get("nc")
    if nc is None:
        nc = _build_program()
        _cache["nc"] = nc

    em = np.ascontiguousarray(np.asarray(emissions, dtype=np.float32))
    tg = np.ascontiguousarray(np.asarray(tags).astype(np.int32))
    tr = np.ascontiguousarray(np.asarray(transitions, dtype=np.float32))
    st = np.ascontiguousarray(np.asarray(start_transitions, dtype=np.float32))
    en = np.ascontiguousarray(np.asarray(end_transitions, dtype=np.float32))
    # masks are all ones for this problem (spec fill: "ones") — unused.

    in_maps = []
    for k in range(_NCORES):
        sl = slice(k * _BL, (k + 1) * _BL)
        in_maps.append(dict(em=em[sl], tags=tg[sl], trans=tr,
                            start_t=st, end_t=en))
    res = run_bass_kernel_spmd(nc, in_maps, list(range(_NCORES)))
    last_results = res
    total = sum(float(r["partial"][0, 0]) for r in res.results)
    return np.float32(total / _B)


# revision 61
# speedup vs baseline: 1.1123x; 1.0297x over previous
"""CRF negative-log-likelihood loss kernel for Trainium2 (8 NeuronCores).

Problem: nn_ConditionalRandomField — B=128, S=512, T=256.
loss = mean_b( log Z_b - score_b ).

Strategy (data-parallel over batch, 16 batches/core):
  * Partition function in exp space, no renormalization (c=1/422
    prescale keeps q in range over 511 steps; ln c re-added at end).
  * The serial scan is split in HALF: a forward recurrence
    qf(s) = e_s (*) (A^T qf(s-1)) over steps 1..255 and a backward
    recurrence xb(s) = e_s (*) (A xb(s+1)) over steps 511..257 run as
    two independent chains that fill each other's engine-latency gaps
    (PE matmul block -> DVE PSUM-multiply round trip). They meet at
    Z_b = qf(255)^T . A xb(256-ish), one matmul + multiply at the end.
    This halves the number of serial rounds (255 instead of 511).
  * u PSUM tiles are double-buffered so the first matmul of each round
    carries only the DVE-data wait — its LDWEIGHTS issues during the
    multiply instead of after it.
  * Emission prep (PE transpose -> ACT exp into the resident eem
    buffer) and the gold-path score (DVE one-hots -> accumulated count
    matmuls with [onehot_next | em_bf16] moving; Pool casts) are
    spread as fine-grained quanta through the scan rounds.

Self-contained: shapes/sharding hardcoded.
"""

import math
import numpy as np

_B, _S, _T = 128, 512, 256
_NCORES = 8
_BL = _B // _NCORES          # 16 batches per core
_NCH = _S // 128             # 4 chunks of 128 steps
_CDEN = 422.0
_LN_CDEN = math.log(_CDEN)
_HALF = 255                  # fwd rounds; bwd does 255 + 1 closing MM

_cache = {}
last_results = None


def _build_program():
    from contextlib import ExitStack

    import concourse.bass as bass
    import concourse.tile as tile
    from concourse import bacc, mybir

    f32 = mybir.dt.float32
    bf16 = mybir.dt.bfloat16
    i32 = mybir.dt.int32
    MUL = mybir.AluOpType.mult
    ADD = mybir.AluOpType.add
    SUB = mybir.AluOpType.subtract
    EQ = mybir.AluOpType.is_equal
    EXP = mybir.ActivationFunctionType.Exp
    LN = mybir.ActivationFunctionType.Ln
    X = mybir.AxisListType.X

    nc = bacc.Bacc("TRN2", target_bir_lowering=False, debug=False,
                   num_devices=_NCORES)

    em_d = nc.dram_tensor("em", [_BL, _S, _T], f32, kind="ExternalInput")
    tags_d = nc.dram_tensor("tags", [_BL, _S], i32, kind="ExternalInput")
    trans_d = nc.dram_tensor("trans", [_T, _T], f32, kind="ExternalInput")
    start_d = nc.dram_tensor("start_t", [_T], f32, kind="ExternalInput")
    end_d = nc.dram_tensor("end_t", [_T], f32, kind="ExternalInput")
    part_d = nc.dram_tensor("partial", [1, 1], f32, kind="ExternalOutput")

    with tile.TileContext(nc) as tc, ExitStack() as ctx:
        singles = ctx.enter_context(tc.tile_pool(name="singles", bufs=1))

        # ---- constants ----
        iota_i = singles.tile([128, _T], i32)
        nc.gpsimd.iota(iota_i[:], pattern=[[1, _T]], base=0, channel_multiplier=0)
        iota_f = singles.tile([128, _T], f32)
        nc.vector.tensor_copy(iota_f[:], iota_i[:])
        pidx_i = singles.tile([128, 1], i32)
        nc.gpsimd.iota(pidx_i[:], pattern=[[0, 1]], base=0, channel_multiplier=1)
        pidx_f = singles.tile([128, 1], f32)
        nc.vector.tensor_copy(pidx_f[:], pidx_i[:])
        ident = singles.tile([128, 128], f32)
        nc.vector.tensor_scalar(out=ident[:], in0=iota_f[:, 0:128],
                                scalar1=pidx_f[:, 0:1], scalar2=None, op0=EQ)
        ones_f = singles.tile([128, 1], f32)
        nc.vector.memset(ones_f[:], 1.0)
        ones_row = singles.tile([1, 128], f32)
        nc.vector.memset(ones_row[:], 1.0)
        lnc_neg = singles.tile([128, 1], f32)
        nc.vector.memset(lnc_neg[:], -_LN_CDEN)
        lnc_pos = singles.tile([128, 1], f32)
        nc.vector.memset(lnc_pos[:], _LN_CDEN)

        # ---- pools ----
        empool = ctx.enter_context(tc.tile_pool(name="em", bufs=33))
        ohpool = ctx.enter_context(tc.tile_pool(name="oh", bufs=4))
        mvpool = ctx.enter_context(tc.tile_pool(name="mv", bufs=4))
        tppool = ctx.enter_context(
            tc.tile_pool(name="tp", bufs=2, space="PSUM"))
        cpool = ctx.enter_context(
            tc.tile_pool(name="cps", bufs=1, space="PSUM"))
        ufpool = ctx.enter_context(
            tc.tile_pool(name="uf", bufs=2, space="PSUM"))
        ubpool = ctx.enter_context(
            tc.tile_pool(name="ub", bufs=2, space="PSUM"))
        qpool = ctx.enter_context(tc.tile_pool(name="q", bufs=2))

        # ---- transition params (A = exp(trans)) ----
        tr_sb = singles.tile([128, 2, _T], f32)
        nc.sync.dma_start(tr_sb[:], trans_d[:].rearrange("(h p) j -> p h j", p=128))
        etrans = singles.tile([128, 2, _T], bf16)
        nc.scalar.activation(etrans[:, 0, :], tr_sb[:, 0, :], EXP, bias=0.0, scale=1.0)
        nc.scalar.activation(etrans[:, 1, :], tr_sb[:, 1, :], EXP, bias=0.0, scale=1.0)
        # transposed A for the backward chain: eAT[p, hj, i] = exp(trans[i, hj*128+p])
        eAT = singles.tile([128, 2, _T], bf16)
        for hi in range(2):
            for hj in range(2):
                tpx = tppool.tile([128, 128], f32, tag="tp")
                nc.tensor.transpose(tpx[:], tr_sb[:, hi, hj * 128:(hj + 1) * 128],
                                    ident[:])
                nc.scalar.activation(eAT[:, hj, hi * 128:(hi + 1) * 128], tpx[:],
                                     EXP, bias=0.0, scale=1.0)

        st_pc = singles.tile([128, 2], f32)
        nc.sync.dma_start(st_pc[:], start_d[:].rearrange("(h p) -> p h", p=128))
        estart = singles.tile([128, 2], f32)  # exp(start)/c
        nc.scalar.activation(estart[:], st_pc[:], EXP, bias=lnc_pos[:, 0:1], scale=1.0)
        en_pc = singles.tile([128, 2], f32)
        nc.sync.dma_start(en_pc[:], end_d[:].rearrange("(h p) -> p h", p=128))
        eend = singles.tile([128, 2], f32)
        nc.scalar.activation(eend[:], en_pc[:], EXP, bias=0.0, scale=1.0)
        pidx2_i = singles.tile([128, 2], i32)
        nc.gpsimd.iota(pidx2_i[:], pattern=[[128, 2]], base=0, channel_multiplier=1)
        pidx2_f = singles.tile([128, 2], f32)
        nc.vector.tensor_copy(pidx2_f[:], pidx2_i[:])

        # ---- tag columns (tiles; DMAs emitted after the em bootstrap) ----
        tcol_i = singles.tile([128, _BL, _NCH], i32)
        tcol2_i = singles.tile([128, _BL, _NCH], i32)
        nc.gpsimd.memset(tcol2_i[:], -1)
        tcol_f = singles.tile([128, _BL, _NCH], f32)
        tcol2_f = singles.tile([128, _BL, _NCH], f32)
        tf_i = singles.tile([1, _BL], i32)
        tl_i = singles.tile([1, _BL], i32)
        oh_se = singles.tile([128, 2, 2, _BL], f32)

        def emit_tag_dmas():
            nc.sync.dma_start(tcol_i[:],
                              tags_d[:].rearrange("b (c p) -> p b c", p=128))
            for b in range(_BL):
                nc.sync.dma_start(
                    tcol2_i[:, b, 0:_NCH - 1],
                    tags_d[b, 1:1 + 128 * (_NCH - 1)].rearrange("(c p) -> p c", p=128))
                nc.sync.dma_start(
                    tcol2_i[0:127, b, _NCH - 1:_NCH],
                    tags_d[b, 1 + 128 * (_NCH - 1):_S].rearrange("(c p) -> p c", p=127))
            nc.sync.dma_start(tf_i[:], tags_d[:, 0:1].rearrange("b o -> o b"))
            nc.sync.dma_start(tl_i[:], tags_d[:, _S - 1:_S].rearrange("b o -> o b"))

        rnum = singles.tile([128, 6], f32)
        # one eem tile per 128-step chunk so a scan round's multiply only
        # depends on its own chunk's exp writes (Tile's range tracking
        # would otherwise serialize every round behind every exp)
        eem0 = singles.tile([128, 2, _BL, 128], bf16)
        eem1 = singles.tile([128, 2, _BL, 128], bf16)
        eem2 = singles.tile([128, 2, _BL, 128], bf16)
        eem3 = singles.tile([128, 2, _BL, 128], bf16)
        eem_t = [eem0, eem1, eem2, eem3]
        c_ps = cpool.tile([128, 2, 2 * _T], f32)

        # ---------- prep pieces ----------
        emt = {}

        def dma_chunk_pair(b, grp, eng):
            """One DMA loads a chunk pair for batch b: grp 0 -> chunks
            {0, 3} (the bootstrap pair, via a stride-3 chunk slice),
            grp 1 -> chunks {1, 2}."""
            t = empool.tile([128, 2, _T], f32, tag="emt")
            src = em_d[b, :, :].rearrange("(c p) t -> p c t", p=128)
            if grp == 0:
                eng.dma_start(t[:], src[:, 0:4:3, :])
                emt[(b, 0)] = (t, 0)
                emt[(b, 3)] = (t, 1)
            else:
                eng.dma_start(t[:], src[:, 1:3, :])
                emt[(b, 1)] = (t, 0)
                emt[(b, 2)] = (t, 1)

        _tp_rr = {"i": 0}

        def tpexp_quanta(b, ch, rotate=False):
            """Transpose both tag halves into one PSUM tile, then a single
            exp writes eem for this (batch, chunk): 3 quanta. In bootstrap
            (rotate=True) PSUM slots rotate across tp/uf/ub pools."""
            st = {}

            def tp(jh):
                def go():
                    if jh == 0:
                        if rotate:
                            pool, tg = ((tppool, "tp"), (ufpool, "uf"),
                                        (ubpool, "ub"))[_tp_rr["i"] % 3]
                            _tp_rr["i"] += 1
                        else:
                            pool, tg = tppool, "tp"
                        tpt = pool.tile([128, 2, 128], f32, tag=tg)
                        st["t"] = tpt
                    et, ec = emt[(b, ch)]
                    nc.tensor.transpose(st["t"][:, jh, :],
                                        et[:, ec, jh * 128:(jh + 1) * 128],
                                        ident[:])
                return go

            def ex():
                nc.scalar.activation(
                    eem_t[ch][:, :, b, :], st.pop("t"), EXP,
                    bias=lnc_neg[:, 0:1], scale=1.0)

            return [tp(0), tp(1), ex]

        _cmm_state = {"n": 0}

        def score_quanta(b, ch):
            """One-hots + cast + count-matmuls for one (batch, chunk)."""
            st = {}

            def g_oh1():
                oh1 = ohpool.tile([128, _T], bf16, tag="oh1")
                st["oh1"] = oh1
                nc.vector.tensor_scalar(out=oh1[:], in0=iota_f[:],
                                        scalar1=tcol_f[:, b, ch:ch + 1],
                                        scalar2=None, op0=EQ)

            def g_mv():
                mv = mvpool.tile([128, 2 * _T], bf16, tag="mv")
                st["mv"] = mv
                nc.vector.tensor_scalar(out=mv[:, 0:_T], in0=iota_f[:],
                                        scalar1=tcol2_f[:, b, ch:ch + 1],
                                        scalar2=None, op0=EQ)
                et, ec = emt.pop((b, ch))
                nc.gpsimd.tensor_copy(mv[:, _T:2 * _T], et[:, ec, :])

            def g_cmm():
                first = _cmm_state["n"] == 0
                _cmm_state["n"] += 1
                last = _cmm_state["n"] == _BL * _NCH
                oh1, mv = st["oh1"], st["mv"]
                nc.tensor.matmul(c_ps[:, 0, :], oh1[:, 0:128], mv[:],
                                 start=first, stop=last, skip_group_check=True)
                nc.tensor.matmul(c_ps[:, 1, :], oh1[:, 128:256], mv[:],
                                 start=first, stop=last, skip_group_check=True)

            return [g_oh1, g_mv, g_cmm]

        # ---------- bootstrap ----------
        # All em DMAs on the otherwise-idle sync (SP) HWDGE queue; each
        # DMA carries a chunk pair, so 32 DMAs cover all four chunks.
        for b in range(_BL):
            dma_chunk_pair(b, 0, nc.sync)
        for b in range(_BL):
            dma_chunk_pair(b, 1, nc.sync)
        emit_tag_dmas()
        # transpose+exp for chunks 0 and 3 (needed before round 1)
        for b in range(_BL):
            for g in tpexp_quanta(b, 0, rotate=True):
                g()
            for g in tpexp_quanta(b, 3, rotate=True):
                g()

        # quanta stream fired through the scan rounds:
        #  - tp/exp for chunks 1,2 over rounds 1..110
        #  - score quanta for all chunks over rounds 1..250
        eem_q = []
        for b in range(_BL):
            eem_q.extend(tpexp_quanta(b, 1))
            eem_q.extend(tpexp_quanta(b, 2))
        def g_tcol_copies():
            nc.vector.tensor_copy(tcol_f[:], tcol_i[:])
            nc.vector.tensor_copy(tcol2_f[:], tcol2_i[:])

        sc_q = [g_tcol_copies]
        for ch in (0, 3, 1, 2):
            for b in range(_BL):
                sc_q.extend(score_quanta(b, ch))

        # ---------- init chains ----------
        qf = qpool.tile([128, 2 * _BL], bf16, tag="qf")
        for jh in range(2):
            nc.vector.tensor_scalar(
                out=qf[:, jh * _BL:(jh + 1) * _BL],
                in0=eem0[:, jh, :, 0],
                scalar1=estart[:, jh:jh + 1], scalar2=None, op0=MUL)
        xb = qpool.tile([128, 2 * _BL], bf16, tag="xb")
        for jh in range(2):
            nc.vector.tensor_scalar(
                out=xb[:, jh * _BL:(jh + 1) * _BL],
                in0=eem3[:, jh, :, 127],
                scalar1=eend[:, jh:jh + 1], scalar2=None, op0=MUL)

        # ---------- scan rounds ----------
        EEMQ_START, EEMQ_END = 5, 120
        SCQ_START, SCQ_END = 35, 250
        ei = si = 0
        for r in range(1, _HALF + 1):
            # forward MMs: uf = A^T qf
            uf = ufpool.tile([128, 2 * _BL], f32, tag="uf")
            for jh in range(2):
                o = uf[:, jh * _BL:(jh + 1) * _BL]
                nc.tensor.matmul(o, etrans[:, 0, jh * 128:(jh + 1) * 128],
                                 qf[:, 0:_BL], start=True, stop=False,
                                 skip_group_check=True)
                nc.tensor.matmul(o, etrans[:, 1, jh * 128:(jh + 1) * 128],
                                 qf[:, _BL:2 * _BL], start=False, stop=True,
                                 skip_group_check=True)
            # backward MMs: ub = A xb
            ub = ubpool.tile([128, 2 * _BL], f32, tag="ub")
            for ih in range(2):
                o = ub[:, ih * _BL:(ih + 1) * _BL]
                nc.tensor.matmul(o, eAT[:, 0, ih * 128:(ih + 1) * 128],
                                 xb[:, 0:_BL], start=True, stop=False,
                                 skip_group_check=True)
                nc.tensor.matmul(o, eAT[:, 1, ih * 128:(ih + 1) * 128],
                                 xb[:, _BL:2 * _BL], start=False, stop=True,
                                 skip_group_check=True)
            # multiplies: qf = uf * e_r ; xb = ub * e_{511-r}
            qf = qpool.tile([128, 2 * _BL], bf16, tag="qf")
            nc.vector.tensor_tensor(out=qf[:], in0=uf[:],
                                    in1=eem_t[r // 128][:, :, :, r % 128], op=MUL)
            xb = qpool.tile([128, 2 * _BL], bf16, tag="xb")
            nc.vector.tensor_tensor(out=xb[:], in0=ub[:],
                                    in1=eem_t[(_S - 1 - r) // 128]
                                    [:, :, :, (_S - 1 - r) % 128], op=MUL)
            # fire prep quanta inside their round windows
            ne = (len(eem_q) * max(0, min(r, EEMQ_END) - EEMQ_START)) \
                // (EEMQ_END - EEMQ_START) - ei
            for _ in range(ne):
                eem_q[ei]()
                ei += 1
            ns = (len(sc_q) * max(0, min(r, SCQ_END) - SCQ_START)) \
                // (SCQ_END - SCQ_START) - si
            for _ in range(ns):
                sc_q[si]()
                si += 1

        # ---------- close: qb(256) = A xb_final ; Z = qf . qb ----------
        ub = ubpool.tile([128, 2 * _BL], f32, tag="ub")
        for ih in range(2):
            o = ub[:, ih * _BL:(ih + 1) * _BL]
            nc.tensor.matmul(o, eAT[:, 0, ih * 128:(ih + 1) * 128],
                             xb[:, 0:_BL], start=True, stop=False,
                             skip_group_check=True)
            nc.tensor.matmul(o, eAT[:, 1, ih * 128:(ih + 1) * 128],
                             xb[:, _BL:2 * _BL], start=False, stop=True,
                             skip_group_check=True)
        fpool = ctx.enter_context(tc.tile_pool(name="f", bufs=1))
        zprod = fpool.tile([128, 2 * _BL], f32)
        nc.vector.tensor_tensor(out=zprod[:], in0=ub[:], in1=qf[:], op=MUL)
        zf = tppool.tile([1, 2 * _BL], f32, tag="tp")
        nc.tensor.matmul(zf[:], ones_f[:], zprod[:], start=True, stop=True,
                         skip_group_check=True)
        zsb = fpool.tile([1, 2 * _BL], f32)
        nc.vector.tensor_copy(zsb[:], zf[:])
        zsum = fpool.tile([1, _BL], f32)
        nc.vector.tensor_tensor(out=zsum[:], in0=zsb[:, 0:_BL],
                                in1=zsb[:, _BL:2 * _BL], op=ADD)
        logz = fpool.tile([1, _BL], f32)
        nc.scalar.activation(logz[:], zsum[:], LN, bias=0.0, scale=1.0)
        nc.vector.tensor_scalar(out=logz[:], in0=logz[:],
                                scalar1=float((_S - 1) * _LN_CDEN), scalar2=None,
                                op0=ADD)

        # ---------- start/end one-hot terms ----------
        tf_f = fpool.tile([1, _BL], f32)
        nc.vector.tensor_copy(tf_f[:], tf_i[:])
        tl_f = fpool.tile([1, _BL], f32)
        nc.vector.tensor_copy(tl_f[:], tl_i[:])
        for k, (srci, par) in enumerate(((tf_f, st_pc), (tl_f, en_pc))):
            se_ps = tppool.tile([128, _BL], f32, tag="tp")
            nc.tensor.matmul(se_ps[:], ones_row[:], srci[:],
                             start=True, stop=True)
            for h in range(2):
                nc.vector.tensor_scalar(out=oh_se[:, k, h, :], in0=se_ps[:],
                                        scalar1=pidx2_f[:, h:h + 1],
                                        scalar2=par[:, h:h + 1],
                                        op0=EQ, op1=MUL)
            nc.vector.tensor_reduce(rnum[:, 4 + k:5 + k],
                                    oh_se[:, k, :, :],
                                    axis=mybir.AxisListType.XY, op=ADD)

        # ---------- numerator finalization ----------
        scpool = ctx.enter_context(tc.tile_pool(name="sc", bufs=2))
        for ih in range(2):
            scr = scpool.tile([128, _T], f32, tag="scr")
            nc.vector.tensor_tensor(out=scr[:], in0=c_ps[:, ih, 0:_T],
                                    in1=tr_sb[:, ih, :], op=MUL)
            nc.vector.tensor_reduce(rnum[:, ih:ih + 1], scr[:], axis=X, op=ADD)
        for ih in range(2):
            scr = scpool.tile([128, 128], f32, tag="scr2")
            nc.vector.tensor_tensor(
                out=scr[:], in0=c_ps[:, ih, _T + ih * 128:_T + (ih + 1) * 128],
                in1=ident[:], op=MUL)
            nc.vector.tensor_reduce(rnum[:, 2 + ih:3 + ih], scr[:], axis=X, op=ADD)

        slz = fpool.tile([1, 1], f32)
        nc.vector.tensor_reduce(slz[:], logz[:], axis=X, op=ADD)
        rsum = fpool.tile([128, 1], f32)
        nc.vector.tensor_reduce(rsum[:], rnum[:], axis=X, op=ADD)
        nsum = tppool.tile([1, 1], f32, tag="tp")
        nc.tensor.matmul(nsum[:], ones_f[:], rsum[:], start=True, stop=True,
                         skip_group_check=True)
        part = fpool.tile([1, 1], f32)
        nc.vector.tensor_tensor(out=part[:], in0=slz[:], in1=nsum[:], op=SUB)
        nc.sync.dma_start(part_d[:], part[:])

    nc.compile()
    return nc


def kernel(emissions, tags, masks=None, start_transitions=None,
           transitions=None, end_transitions=None, **_unused):
    from concourse.bass_utils import run_bass_kernel_spmd

    global last_results
    nc = _cache.get("nc")
    if nc is None:
        nc = _build_program()
        _cache["nc"] = nc

    em = np.ascontiguousarray(np.asarray(emissions, dtype=np.float32))
    tg = np.ascontiguousarray(np.asarray(tags).astype(np.int32))
    tr = np.ascontiguousarray(np.asarray(transitions, dtype=np.float32))
    st = np.ascontiguousarray(np.asarray(start_transitions, dtype=np.float32))
    en = np.ascontiguousarray(np.asarray(end_transitions, dtype=np.float32))
    # masks are all ones for this problem (spec fill: "ones") — unused.

    in_maps = []
    for k in range(_NCORES):
        sl = slice(k * _BL, (k + 1) * _BL)
        in_maps.append(dict(em=em[sl], tags=tg[sl], trans=tr,
                            start_t=st, end_t=en))
    res = run_bass_kernel_spmd(nc, in_maps, list(range(_NCORES)))
    last_results = res
    total = sum(float(r["partial"][0, 0]) for r in res.results)
    return np.float32(total / _B)
